# revision 8
# baseline (speedup 1.0000x reference)
"""MoE block (B=16, C=192, H=W=32, E=8, top-2, 3x3 same-conv experts) on 8 trn2 cores.

Strategy (v2, pixel-stationary):
  - Router + top-2 combine computed on host; conv linearity folds the
    expert mix into ONE conv per sample (combined weights). 2 convs/core.
  - Matmul formulation: out[M=128 pixels, N=192 out-ch] with the pixel
    window as the STATIONARY operand and the weights as the MOVING
    operand. Streamed rows per matmul = 192 (vs 512 in the out-ch-major
    form), and M is always a full 128, so total streamed rows hit the
    128x128 packing floor.
  - Contraction (9 taps x 192 in-ch = 1728) is covered by 14 K-chunks
    per (block): 9 full-K chunks for ch 0-127 (one per tap), then
    ch 128-191 packed two-taps-per-chunk using image tiles whose upper
    64 partitions hold the SAME channels pre-shifted by the inter-tap
    pixel offset (d=34 for vertical tap pairs, d=1 horizontal), so one
    AP base serves both halves. 3 pairs via d=34, 1 pair via d=1, plus
    one K=64 single for the leftover tap.
  - 2 samples x 8 pixel-blocks x 14 chunks = 224 matmuls of 192 rows.
  - PE is kept continuously busy from ~0.5us with warmup matmuls sized
    to bridge until the first input DMA lands (an idle gap resets the
    p-state ramp).
  - Input DMAs on SP/ACT (HWDGE) + DVE; output DMAs on Pool (SWDGE) to
    keep HWDGE free; host pre-assembles shifted/duplicated tiles so
    every DMA is a single contiguous transfer.
"""

import numpy as np

B, C, H, W = 16, 192, 32, 32
E, TOPK = 8, 2
NCORES = 8
S = B // NCORES          # samples per core
PW = W + 2               # padded width 34
PP = PW * PW             # padded pixels 1156
NB = 8                   # pixel blocks per sample (4 rows x 32 cols = 128 px)
NCH = 14                 # K-chunks per block
TAPS = [(t // 3, t % 3) for t in range(9)]

_cache = {}


def _build_module():
    import concourse.tile as tile
    from concourse import bacc, mybir

    f32 = mybir.dt.float32
    bf16 = mybir.dt.bfloat16

    nc = bacc.Bacc("TRN2", target_bir_lowering=False, debug=False, num_devices=NCORES)
    ta_d = nc.dram_tensor("ta", [S, 128, PP], bf16, kind="ExternalInput")
    tb34_d = nc.dram_tensor("tb34", [S, 128, PP], bf16, kind="ExternalInput")
    tb1_d = nc.dram_tensor("tb1", [S, 128, PP], bf16, kind="ExternalInput")
    w_d = nc.dram_tensor("w", [S, 128, NCH * C], bf16, kind="ExternalInput")
    out_d = nc.dram_tensor("out", [S, 128, NB, C], f32, kind="ExternalOutput")

    with tile.TileContext(nc) as tc:
        with (
            tc.tile_pool(name="img", bufs=1) as img,
            tc.tile_pool(name="win", bufs=1) as win,
            tc.tile_pool(name="cst", bufs=1) as cst,
            tc.tile_pool(name="ps", bufs=8, space="PSUM") as ps,
            tc.tile_pool(name="oev", bufs=4) as oev,
        ):
            Ta, T34, T1, Wt = {}, {}, {}, {}

            # --- input DMAs -------------------------------------------------
            # All inputs on SP (HWDGE) in consumption order, except the two
            # halves of sample-0's Ta image which ride Pool's SWDGE so their
            # descriptor generation runs in parallel with SP's HWDGE queue.
            # The shared DMA_ENGINES device serializes transfers, so request
            # order == arrival order.
            wt0 = win.tile([128, NCH * C], bf16, name="W_0", tag="W_0")
            Wt[0] = wt0
            wt1 = win.tile([128, NCH * C], bf16, name="W_1", tag="W_1")
            Wt[1] = wt1
            ta0 = img.tile([128, PP], bf16, name="Ta_0", tag="Ta_0")
            Ta[0] = ta0
            t34_0 = img.tile([128, PP], bf16, name="T34_0", tag="T34_0")
            T34[0] = t34_0
            t1_0 = img.tile([128, PP], bf16, name="T1_0", tag="T1_0")
            T1[0] = t1_0
            ta1 = img.tile([128, PP], bf16, name="Ta_1", tag="Ta_1")
            Ta[1] = ta1
            t34_1 = img.tile([128, PP], bf16, name="T34_1", tag="T34_1")
            T34[1] = t34_1
            t1_1 = img.tile([128, PP], bf16, name="T1_1", tag="T1_1")
            T1[1] = t1_1

            R_A = 22 * PW  # Ta rows 0-21: every window of blocks 0-4
            nc.gpsimd.dma_start(ta0[:, 0:R_A], ta_d[0, :, 0:R_A])
            nc.sync.dma_start(wt0[:, 0 : 4 * C], w_d[0, :, 0 : 4 * C])
            nc.gpsimd.dma_start(ta0[:, R_A:PP], ta_d[0, :, R_A:PP])
            nc.sync.dma_start(wt0[:, 4 * C : 9 * C], w_d[0, :, 4 * C : 9 * C])
            nc.sync.dma_start(t34_0[:], tb34_d[0])
            nc.sync.dma_start(t1_0[:], tb1_d[0])
            nc.sync.dma_start(wt0[:, 9 * C : NCH * C], w_d[0, :, 9 * C : NCH * C])
            nc.gpsimd.dma_start(ta1[:], ta_d[1])
            nc.sync.dma_start(wt1[:, 0 : 9 * C], w_d[1, :, 0 : 9 * C])
            nc.sync.dma_start(t34_1[:], tb34_d[1])
            nc.sync.dma_start(wt1[:, 9 * C : NCH * C], w_d[1, :, 9 * C : NCH * C])
            nc.sync.dma_start(t1_1[:], tb1_d[1])

            # --- PSUM tiles (8 banks, one per pixel block) ------------------
            psb = {}
            for s in range(S):
                for b in range(NB):
                    psb[(s, b)] = ps.tile([128, C], f32, name=f"ps_{s}_{b}",
                                          tag="ps")

            # --- matmul emission helpers ------------------------------------
            def lhsT(s, b, c):
                if c < 9:
                    dy, dx = TAPS[c]
                    v = Ta[s][:].rearrange("p (r q) -> p r q", q=PW)
                    return v[:, 4 * b + dy : 4 * b + dy + 4, dx : dx + 32]
                if c < 12:
                    j = c - 9
                    v = T34[s][:].rearrange("p (r q) -> p r q", q=PW)
                    return v[:, 4 * b : 4 * b + 4, j : j + 32]
                v = T1[s][:].rearrange("p (r q) -> p r q", q=PW)
                if c == 12:
                    return v[:, 4 * b + 2 : 4 * b + 6, 0:32]
                return v[0:64, 4 * b + 2 : 4 * b + 6, 2:34]

            def rhs(s, c):
                if c == 13:
                    return Wt[s][0:64, c * C : (c + 1) * C]
                return Wt[s][:, c * C : (c + 1) * C]

            def mm(s, b, c):
                nc.tensor.matmul(psb[(s, b)][:], lhsT(s, b, c), rhs(s, c),
                                 start=(c == 0), stop=(c == NCH - 1))

            # --- eviction + output DMA --------------------------------------
            stages = {}

            def evict(s, b):
                i = b // 2
                if b % 2 == 0:
                    stages[(s, i)] = oev.tile([128, 2 * C], f32,
                                              name=f"st_{s}_{i}", tag="st")
                dst = stages[(s, i)][:, (b % 2) * C : (b % 2 + 1) * C]
                if b % 2 == 0:
                    nc.vector.tensor_scalar_add(dst, psb[(s, b)][:], 0.0)
                else:
                    nc.scalar.copy(dst, psb[(s, b)][:])
                if b % 2 == 1:
                    st = stages[(s, i)][:].rearrange("p (b m) -> p b m", m=C)
                    if s == S - 1 and b == NB - 1:
                        # tail: last transfer on SP's HWDGE (shorter chain
                        # than SWDGE, and both are idle by now)
                        nc.sync.dma_start(out_d[s, :, 2 * i : 2 * i + 2, :], st)
                    else:
                        nc.gpsimd.dma_start(out_d[s, :, 2 * i : 2 * i + 2, :], st)

            # --- sample 0: chunk-sweep order (matches DMA arrival) ----------
            for c in range(4):            # needs W c0-c3 + Ta rows 0-21
                for b in range(5):
                    mm(0, b, c)
            for c in range(4, 9):         # needs W c4-c8
                for b in range(5):
                    mm(0, b, c)
            for b in range(5, NB):        # needs Ta rows 22-33
                for c in range(9):
                    mm(0, b, c)
            for b in range(NB):           # needs T34, T1, W c9-c13
                for c in range(9, NCH):
                    mm(0, b, c)
                evict(0, b)

            # --- sample 1: block-major ------------------------------------
            for b in range(NB):
                for c in range(NCH):
                    mm(1, b, c)
                evict(1, b)

    nc.compile()
    return nc


def get_module():
    if "nc" not in _cache:
        _cache["nc"] = _build_module()
    return _cache["nc"]


def _route(x, gate_w, gate_b):
    """Replicates the reference router in numpy fp32. Returns combine [B,E]."""
    pooled = x.mean(axis=(2, 3), dtype=np.float32)
    logits = pooled @ gate_w + gate_b
    z = logits - logits.max(axis=-1, keepdims=True)
    ez = np.exp(z)
    w = ez / ez.sum(axis=-1, keepdims=True)
    topi = np.argsort(-w, axis=-1, kind="stable")[:, :TOPK]
    topw = np.take_along_axis(w, topi, axis=-1)
    topw = topw / (topw.sum(-1, keepdims=True) + 1e-10)
    combine = np.zeros((B, E), np.float32)
    np.put_along_axis(combine, topi, topw, axis=-1)
    return combine


def make_in_maps(x, gate_w, gate_b, expert_w, expert_b):
    import ml_dtypes

    bf16 = ml_dtypes.bfloat16
    x = np.ascontiguousarray(np.asarray(x, np.float32))
    gate_w = np.asarray(gate_w, np.float32)
    gate_b = np.asarray(gate_b, np.float32)
    expert_w = np.asarray(expert_w, np.float32)
    expert_b = np.asarray(expert_b, np.float32)

    combine = _route(x, gate_w, gate_b)                       # [B,E]
    Wc = np.einsum("be,eoikl->boikl", combine, expert_w)      # [B,C,C,3,3]
    bc = combine @ expert_b                                   # [B,C]

    # Padded input images: [B, C, 34*34]
    xp = np.zeros((B, C, PW, PW), np.float32)
    xp[:, :, 1 : H + 1, 1 : W + 1] = x
    xp = xp.reshape(B, C, PP).astype(bf16)

    ta = xp[:, 0:128]                                         # [B,128,1156]
    img64 = xp[:, 128:192]                                    # [B,64,1156]

    # ch128-191 duplicated with the upper half pre-shifted (d=34 / d=1)
    tb34 = np.zeros((B, 128, PP), bf16)
    tb34[:, 0:64] = img64
    tb34[:, 64:128, 0 : PP - PW] = img64[:, :, PW:]
    tb1 = np.zeros((B, 128, PP), bf16)
    tb1[:, 0:64] = img64
    tb1[:, 64:128, 0 : PP - 1] = img64[:, :, 1:]

    # Moving weights, one [128, 192] slab per K-chunk:
    #   WT[b, t, i, o] = Wc[b, o, i, dy, dx]
    WT = Wc.transpose(0, 3, 4, 2, 1).reshape(B, 9, C, C)
    w = np.zeros((B, 128, NCH * C), np.float32)
    for c in range(9):                        # ch 0-127, tap c
        w[:, :, c * C : (c + 1) * C] = WT[:, c, 0:128, :]
    for j in range(3):                        # pairs (0,j)+(1,j), d=34
        c = 9 + j
        w[:, 0:64, c * C : (c + 1) * C] = WT[:, j, 128:192, :]
        w[:, 64:128, c * C : (c + 1) * C] = WT[:, 3 + j, 128:192, :]
    w[:, 0:64, 12 * C : 13 * C] = WT[:, 6, 128:192, :]        # pair (2,0)+(2,1), d=1
    w[:, 64:128, 12 * C : 13 * C] = WT[:, 7, 128:192, :]
    w[:, 0:64, 13 * C : 14 * C] = WT[:, 8, 128:192, :]        # single (2,2), K=64
    w = w.astype(bf16)

    in_maps = []
    for cidx in range(NCORES):
        b0 = S * cidx
        in_maps.append(
            {
                "ta": np.ascontiguousarray(ta[b0 : b0 + S]),
                "tb34": np.ascontiguousarray(tb34[b0 : b0 + S]),
                "tb1": np.ascontiguousarray(tb1[b0 : b0 + S]),
                "w": np.ascontiguousarray(w[b0 : b0 + S]),
            }
        )
    return in_maps, bc


def postprocess(dev_out, bc_rows):
    """[S, 128, NB, 192] device tensor + per-sample bias rows -> [S,C,H,W]."""
    dev = np.asarray(dev_out, np.float32)
    out = dev.transpose(0, 3, 2, 1).reshape(S, C, NB, 4, 32).reshape(S, C, H, W)
    return out + bc_rows[:, :, None, None]


def kernel(x, gate_w, gate_b, expert_w, expert_b):
    from concourse.bass_utils import run_bass_kernel_spmd

    nc = get_module()
    in_maps, bc = make_in_maps(x, gate_w, gate_b, expert_w, expert_b)
    res = run_bass_kernel_spmd(nc, in_maps, core_ids=list(range(NCORES)))
    out = np.empty((B, C, H, W), np.float32)
    for c in range(NCORES):
        b0 = S * c
        out[b0 : b0 + S] = postprocess(res.results[c]["out"], bc[b0 : b0 + S])
    return out


# revision 10
# speedup vs baseline: 1.1719x; 1.1719x over previous
"""MoE block (B=16, C=192, H=W=32, E=8, top-2, 3x3 same-conv experts) on 8 trn2 cores.

Strategy (v2, pixel-stationary):
  - Router + top-2 combine computed on host; conv linearity folds the
    expert mix into ONE conv per sample (combined weights). 2 convs/core.
  - Matmul formulation: out[M=128 pixels, N=192 out-ch] with the pixel
    window as the STATIONARY operand and the weights as the MOVING
    operand. Streamed rows per matmul = 192 (vs 512 in the out-ch-major
    form), and M is always a full 128, so total streamed rows hit the
    128x128 packing floor.
  - Contraction (9 taps x 192 in-ch = 1728) is covered by 14 K-chunks
    per (block): 9 full-K chunks for ch 0-127 (one per tap), then
    ch 128-191 packed two-taps-per-chunk using image tiles whose upper
    64 partitions hold the SAME channels pre-shifted by the inter-tap
    pixel offset (d=34 for vertical tap pairs, d=1 horizontal), so one
    AP base serves both halves. 3 pairs via d=34, 1 pair via d=1, plus
    one K=64 single for the leftover tap.
  - 2 samples x 8 pixel-blocks x 14 chunks = 224 matmuls of 192 rows.
  - PE is kept continuously busy from ~0.5us with warmup matmuls sized
    to bridge until the first input DMA lands (an idle gap resets the
    p-state ramp).
  - Input DMAs on SP/ACT (HWDGE) + DVE; output DMAs on Pool (SWDGE) to
    keep HWDGE free; host pre-assembles shifted/duplicated tiles so
    every DMA is a single contiguous transfer.
"""

import numpy as np

B, C, H, W = 16, 192, 32, 32
E, TOPK = 8, 2
NCORES = 8
S = B // NCORES          # samples per core
PW = W + 2               # padded width 34
PP = PW * PW             # padded pixels 1156
NB = 8                   # pixel blocks per sample (4 rows x 32 cols = 128 px)
NCH = 14                 # K-chunks per block
TAPS = [(t // 3, t % 3) for t in range(9)]

# Warmup matmul row-counts: first runs at the LOW p-state, rest at MID.
WARMUP_NS = [192] * 17

_cache = {}


def _build_module():
    import concourse.tile as tile
    from concourse import bacc, mybir

    f32 = mybir.dt.float32
    bf16 = mybir.dt.bfloat16

    nc = bacc.Bacc("TRN2", target_bir_lowering=False, debug=False, num_devices=NCORES)
    ta_d = nc.dram_tensor("ta", [S, 128, PP], bf16, kind="ExternalInput")
    tb34_d = nc.dram_tensor("tb34", [S, 128, PP], bf16, kind="ExternalInput")
    tb1_d = nc.dram_tensor("tb1", [S, 128, PP], bf16, kind="ExternalInput")
    w_d = nc.dram_tensor("w", [S, 128, NCH * C], bf16, kind="ExternalInput")
    out_d = nc.dram_tensor("out", [S, 128, NB, C], f32, kind="ExternalOutput")

    with tile.TileContext(nc) as tc:
        with (
            tc.tile_pool(name="img", bufs=1) as img,
            tc.tile_pool(name="win", bufs=1) as win,
            tc.tile_pool(name="cst", bufs=1) as cst,
            tc.tile_pool(name="ps", bufs=8, space="PSUM") as ps,
            tc.tile_pool(name="oev", bufs=4) as oev,
        ):
            Ta, T34, T1, Wt = {}, {}, {}, {}

            # --- input DMAs -------------------------------------------------
            # All inputs on SP (HWDGE) in consumption order, except the two
            # halves of sample-0's Ta image which ride Pool's SWDGE so their
            # descriptor generation runs in parallel with SP's HWDGE queue.
            # The shared DMA_ENGINES device serializes transfers, so request
            # order == arrival order.
            wt0 = win.tile([128, NCH * C], bf16, name="W_0", tag="W_0")
            Wt[0] = wt0
            wt1 = win.tile([128, NCH * C], bf16, name="W_1", tag="W_1")
            Wt[1] = wt1
            ta0 = img.tile([128, PP], bf16, name="Ta_0", tag="Ta_0")
            Ta[0] = ta0
            t34_0 = img.tile([128, PP], bf16, name="T34_0", tag="T34_0")
            T34[0] = t34_0
            t1_0 = img.tile([128, PP], bf16, name="T1_0", tag="T1_0")
            T1[0] = t1_0
            ta1 = img.tile([128, PP], bf16, name="Ta_1", tag="Ta_1")
            Ta[1] = ta1
            t34_1 = img.tile([128, PP], bf16, name="T34_1", tag="T34_1")
            T34[1] = t34_1
            t1_1 = img.tile([128, PP], bf16, name="T1_1", tag="T1_1")
            T1[1] = t1_1

            R_A = 22 * PW  # Ta rows 0-21: every window of blocks 0-4
            nc.gpsimd.dma_start(ta0[:, 0:R_A], ta_d[0, :, 0:R_A])
            nc.sync.dma_start(wt0[:, 0 : 4 * C], w_d[0, :, 0 : 4 * C])
            nc.gpsimd.dma_start(ta0[:, R_A:PP], ta_d[0, :, R_A:PP])
            nc.sync.dma_start(wt0[:, 4 * C : 9 * C], w_d[0, :, 4 * C : 9 * C])
            nc.sync.dma_start(t34_0[:], tb34_d[0])
            nc.sync.dma_start(t1_0[:], tb1_d[0])
            nc.sync.dma_start(wt0[:, 9 * C : NCH * C], w_d[0, :, 9 * C : NCH * C])
            nc.gpsimd.dma_start(ta1[:], ta_d[1])
            nc.sync.dma_start(wt1[:, 0 : 9 * C], w_d[1, :, 0 : 9 * C])
            nc.sync.dma_start(t34_1[:], tb34_d[1])
            nc.sync.dma_start(wt1[:, 9 * C : NCH * C], w_d[1, :, 9 * C : NCH * C])
            nc.sync.dma_start(t1_1[:], tb1_d[1])

            # --- PSUM tiles (8 banks, one per pixel block) ------------------
            psb = {}
            for s in range(S):
                for b in range(NB):
                    psb[(s, b)] = ps.tile([128, C], f32, name=f"ps_{s}_{b}",
                                          tag="ps")

            # --- PE warmup on zeros: keeps the PE busy (and the p-state
            # ramp anchored) until the first real inputs arrive. ------------
            scr = cst.tile([128, C], bf16, name="scr", tag="scr")
            nc.vector.memset(scr[:], 0.0)
            for n in WARMUP_NS:
                nc.tensor.matmul(psb[(0, 0)][:, 0:n], scr[:, 0:128], scr[:, 0:n],
                                 start=True, stop=True, skip_group_check=True)

            # --- matmul emission helpers ------------------------------------
            def lhsT(s, b, c):
                if c < 9:
                    dy, dx = TAPS[c]
                    v = Ta[s][:].rearrange("p (r q) -> p r q", q=PW)
                    return v[:, 4 * b + dy : 4 * b + dy + 4, dx : dx + 32]
                if c < 12:
                    j = c - 9
                    v = T34[s][:].rearrange("p (r q) -> p r q", q=PW)
                    return v[:, 4 * b : 4 * b + 4, j : j + 32]
                v = T1[s][:].rearrange("p (r q) -> p r q", q=PW)
                if c == 12:
                    return v[:, 4 * b + 2 : 4 * b + 6, 0:32]
                return v[0:64, 4 * b + 2 : 4 * b + 6, 2:34]

            def rhs(s, c):
                if c == 13:
                    return Wt[s][0:64, c * C : (c + 1) * C]
                return Wt[s][:, c * C : (c + 1) * C]

            def mm(s, b, c):
                nc.tensor.matmul(psb[(s, b)][:], lhsT(s, b, c), rhs(s, c),
                                 start=(c == 0), stop=(c == NCH - 1))

            # --- eviction + output DMA --------------------------------------
            stages = {}

            def evict(s, b):
                i = b // 2
                if b % 2 == 0:
                    stages[(s, i)] = oev.tile([128, 2 * C], f32,
                                              name=f"st_{s}_{i}", tag="st")
                dst = stages[(s, i)][:, (b % 2) * C : (b % 2 + 1) * C]
                if b % 2 == 0:
                    nc.vector.tensor_scalar_add(dst, psb[(s, b)][:], 0.0)
                else:
                    nc.scalar.copy(dst, psb[(s, b)][:])
                if b % 2 == 1:
                    st = stages[(s, i)][:].rearrange("p (b m) -> p b m", m=C)
                    if s == S - 1 and b == NB - 1:
                        # tail: last transfer on SP's HWDGE (shorter chain
                        # than SWDGE, and both are idle by now)
                        nc.sync.dma_start(out_d[s, :, 2 * i : 2 * i + 2, :], st)
                    else:
                        nc.gpsimd.dma_start(out_d[s, :, 2 * i : 2 * i + 2, :], st)

            # --- sample 0: chunk-sweep order (matches DMA arrival) ----------
            for c in range(4):            # needs W c0-c3 + Ta rows 0-21
                for b in range(5):
                    mm(0, b, c)
            for c in range(4, 9):         # needs W c4-c8
                for b in range(5):
                    mm(0, b, c)
            for b in range(5, NB):        # needs Ta rows 22-33
                for c in range(9):
                    mm(0, b, c)
            for b in range(NB):           # needs T34, T1, W c9-c13
                for c in range(9, NCH):
                    mm(0, b, c)
                evict(0, b)

            # --- sample 1: block-major ------------------------------------
            for b in range(NB):
                for c in range(NCH):
                    mm(1, b, c)
                evict(1, b)

    nc.compile()
    return nc


def get_module():
    if "nc" not in _cache:
        _cache["nc"] = _build_module()
    return _cache["nc"]


def _route(x, gate_w, gate_b):
    """Replicates the reference router in numpy fp32. Returns combine [B,E]."""
    pooled = x.mean(axis=(2, 3), dtype=np.float32)
    logits = pooled @ gate_w + gate_b
    z = logits - logits.max(axis=-1, keepdims=True)
    ez = np.exp(z)
    w = ez / ez.sum(axis=-1, keepdims=True)
    topi = np.argsort(-w, axis=-1, kind="stable")[:, :TOPK]
    topw = np.take_along_axis(w, topi, axis=-1)
    topw = topw / (topw.sum(-1, keepdims=True) + 1e-10)
    combine = np.zeros((B, E), np.float32)
    np.put_along_axis(combine, topi, topw, axis=-1)
    return combine


def make_in_maps(x, gate_w, gate_b, expert_w, expert_b):
    import ml_dtypes

    bf16 = ml_dtypes.bfloat16
    x = np.ascontiguousarray(np.asarray(x, np.float32))
    gate_w = np.asarray(gate_w, np.float32)
    gate_b = np.asarray(gate_b, np.float32)
    expert_w = np.asarray(expert_w, np.float32)
    expert_b = np.asarray(expert_b, np.float32)

    combine = _route(x, gate_w, gate_b)                       # [B,E]
    Wc = np.einsum("be,eoikl->boikl", combine, expert_w)      # [B,C,C,3,3]
    bc = combine @ expert_b                                   # [B,C]

    # Padded input images: [B, C, 34*34]
    xp = np.zeros((B, C, PW, PW), np.float32)
    xp[:, :, 1 : H + 1, 1 : W + 1] = x
    xp = xp.reshape(B, C, PP).astype(bf16)

    ta = xp[:, 0:128]                                         # [B,128,1156]
    img64 = xp[:, 128:192]                                    # [B,64,1156]

    # ch128-191 duplicated with the upper half pre-shifted (d=34 / d=1)
    tb34 = np.zeros((B, 128, PP), bf16)
    tb34[:, 0:64] = img64
    tb34[:, 64:128, 0 : PP - PW] = img64[:, :, PW:]
    tb1 = np.zeros((B, 128, PP), bf16)
    tb1[:, 0:64] = img64
    tb1[:, 64:128, 0 : PP - 1] = img64[:, :, 1:]

    # Moving weights, one [128, 192] slab per K-chunk:
    #   WT[b, t, i, o] = Wc[b, o, i, dy, dx]
    WT = Wc.transpose(0, 3, 4, 2, 1).reshape(B, 9, C, C)
    w = np.zeros((B, 128, NCH * C), np.float32)
    for c in range(9):                        # ch 0-127, tap c
        w[:, :, c * C : (c + 1) * C] = WT[:, c, 0:128, :]
    for j in range(3):                        # pairs (0,j)+(1,j), d=34
        c = 9 + j
        w[:, 0:64, c * C : (c + 1) * C] = WT[:, j, 128:192, :]
        w[:, 64:128, c * C : (c + 1) * C] = WT[:, 3 + j, 128:192, :]
    w[:, 0:64, 12 * C : 13 * C] = WT[:, 6, 128:192, :]        # pair (2,0)+(2,1), d=1
    w[:, 64:128, 12 * C : 13 * C] = WT[:, 7, 128:192, :]
    w[:, 0:64, 13 * C : 14 * C] = WT[:, 8, 128:192, :]        # single (2,2), K=64
    w = w.astype(bf16)

    in_maps = []
    for cidx in range(NCORES):
        b0 = S * cidx
        in_maps.append(
            {
                "ta": np.ascontiguousarray(ta[b0 : b0 + S]),
                "tb34": np.ascontiguousarray(tb34[b0 : b0 + S]),
                "tb1": np.ascontiguousarray(tb1[b0 : b0 + S]),
                "w": np.ascontiguousarray(w[b0 : b0 + S]),
            }
        )
    return in_maps, bc


def postprocess(dev_out, bc_rows):
    """[S, 128, NB, 192] device tensor + per-sample bias rows -> [S,C,H,W]."""
    dev = np.asarray(dev_out, np.float32)
    out = dev.transpose(0, 3, 2, 1).reshape(S, C, NB, 4, 32).reshape(S, C, H, W)
    return out + bc_rows[:, :, None, None]


def kernel(x, gate_w, gate_b, expert_w, expert_b):
    from concourse.bass_utils import run_bass_kernel_spmd

    nc = get_module()
    in_maps, bc = make_in_maps(x, gate_w, gate_b, expert_w, expert_b)
    res = run_bass_kernel_spmd(nc, in_maps, core_ids=list(range(NCORES)))
    out = np.empty((B, C, H, W), np.float32)
    for c in range(NCORES):
        b0 = S * c
        out[b0 : b0 + S] = postprocess(res.results[c]["out"], bc[b0 : b0 + S])
    return out


# revision 12
# speedup vs baseline: 1.1868x; 1.0127x over previous
"""MoE block (B=16, C=192, H=W=32, E=8, top-2, 3x3 same-conv experts) on 8 trn2 cores.

Strategy (v2, pixel-stationary):
  - Router + top-2 combine computed on host; conv linearity folds the
    expert mix into ONE conv per sample (combined weights). 2 convs/core.
  - Matmul formulation: out[M=128 pixels, N=192 out-ch] with the pixel
    window as the STATIONARY operand and the weights as the MOVING
    operand. Streamed rows per matmul = 192 (vs 512 in the out-ch-major
    form), and M is always a full 128, so total streamed rows hit the
    128x128 packing floor.
  - Contraction (9 taps x 192 in-ch = 1728) is covered by 14 K-chunks
    per (block): 9 full-K chunks for ch 0-127 (one per tap), then
    ch 128-191 packed two-taps-per-chunk using image tiles whose upper
    64 partitions hold the SAME channels pre-shifted by the inter-tap
    pixel offset (d=34 for vertical tap pairs, d=1 horizontal), so one
    AP base serves both halves. 3 pairs via d=34, 1 pair via d=1, plus
    one K=64 single for the leftover tap.
  - 2 samples x 8 pixel-blocks x 14 chunks = 224 matmuls of 192 rows.
  - PE is kept continuously busy from ~0.5us with warmup matmuls sized
    to bridge until the first input DMA lands (an idle gap resets the
    p-state ramp).
  - Input DMAs on SP/ACT (HWDGE) + DVE; output DMAs on Pool (SWDGE) to
    keep HWDGE free; host pre-assembles shifted/duplicated tiles so
    every DMA is a single contiguous transfer.
"""

import numpy as np

B, C, H, W = 16, 192, 32, 32
E, TOPK = 8, 2
NCORES = 8
S = B // NCORES          # samples per core
PW = W + 2               # padded width 34
PP = PW * PW             # padded pixels 1156
NB = 8                   # pixel blocks per sample (4 rows x 32 cols = 128 px)
NCH = 14                 # K-chunks per block
TAPS = [(t // 3, t % 3) for t in range(9)]

# Warmup matmul row-counts: first runs at the LOW p-state, rest at MID.
WARMUP_NS = [192] * 17

_cache = {}


def _build_module():
    import concourse.tile as tile
    from concourse import bacc, mybir

    f32 = mybir.dt.float32
    bf16 = mybir.dt.bfloat16

    nc = bacc.Bacc("TRN2", target_bir_lowering=False, debug=False, num_devices=NCORES)
    ta_d = nc.dram_tensor("ta", [S, 128, PP], bf16, kind="ExternalInput")
    tb34_d = nc.dram_tensor("tb34", [S, 128, PP], bf16, kind="ExternalInput")
    tb1_d = nc.dram_tensor("tb1", [S, 128, PP], bf16, kind="ExternalInput")
    w_d = nc.dram_tensor("w", [S, 128, NCH * C], bf16, kind="ExternalInput")
    out_d = nc.dram_tensor("out", [S, 128, NB, C], f32, kind="ExternalOutput")

    with tile.TileContext(nc) as tc:
        with (
            tc.tile_pool(name="img", bufs=1) as img,
            tc.tile_pool(name="win", bufs=1) as win,
            tc.tile_pool(name="cst", bufs=1) as cst,
            tc.tile_pool(name="ps", bufs=8, space="PSUM") as ps,
            tc.tile_pool(name="oev", bufs=4) as oev,
        ):
            Ta, T34, T1, Wt = {}, {}, {}, {}

            # --- input DMAs -------------------------------------------------
            # All inputs on SP (HWDGE) in consumption order, except the two
            # halves of sample-0's Ta image which ride Pool's SWDGE so their
            # descriptor generation runs in parallel with SP's HWDGE queue.
            # The shared DMA_ENGINES device serializes transfers, so request
            # order == arrival order.
            wt0 = win.tile([128, NCH * C], bf16, name="W_0", tag="W_0")
            Wt[0] = wt0
            wt1 = win.tile([128, NCH * C], bf16, name="W_1", tag="W_1")
            Wt[1] = wt1
            ta0 = img.tile([128, PP], bf16, name="Ta_0", tag="Ta_0")
            Ta[0] = ta0
            t34_0 = img.tile([128, PP], bf16, name="T34_0", tag="T34_0")
            T34[0] = t34_0
            t1_0 = img.tile([128, PP], bf16, name="T1_0", tag="T1_0")
            T1[0] = t1_0
            ta1 = img.tile([128, PP], bf16, name="Ta_1", tag="Ta_1")
            Ta[1] = ta1
            t34_1 = img.tile([128, PP], bf16, name="T34_1", tag="T34_1")
            T34[1] = t34_1
            t1_1 = img.tile([128, PP], bf16, name="T1_1", tag="T1_1")
            T1[1] = t1_1

            R_A = 22 * PW  # Ta rows 0-21: every window of blocks 0-4
            nc.gpsimd.dma_start(ta0[:, 0:R_A], ta_d[0, :, 0:R_A])
            nc.sync.dma_start(wt0[:, 0 : 4 * C], w_d[0, :, 0 : 4 * C])
            nc.gpsimd.dma_start(ta0[:, R_A:PP], ta_d[0, :, R_A:PP])
            nc.sync.dma_start(wt0[:, 4 * C : 9 * C], w_d[0, :, 4 * C : 9 * C])
            nc.sync.dma_start(t34_0[:], tb34_d[0])
            nc.sync.dma_start(t1_0[:], tb1_d[0])
            nc.sync.dma_start(wt0[:, 9 * C : NCH * C], w_d[0, :, 9 * C : NCH * C])
            nc.gpsimd.dma_start(ta1[:], ta_d[1])
            nc.sync.dma_start(wt1[:, 0 : 9 * C], w_d[1, :, 0 : 9 * C])
            nc.sync.dma_start(t34_1[:], tb34_d[1])
            nc.sync.dma_start(wt1[:, 9 * C : NCH * C], w_d[1, :, 9 * C : NCH * C])
            nc.sync.dma_start(t1_1[:], tb1_d[1])

            # --- PSUM tiles (8 banks, one per pixel block) ------------------
            psb = {}
            for s in range(S):
                for b in range(NB):
                    psb[(s, b)] = ps.tile([128, C], f32, name=f"ps_{s}_{b}",
                                          tag="ps")

            # --- PE warmup on zeros: keeps the PE busy (and the p-state
            # ramp anchored) until the first real inputs arrive. ------------
            scr = cst.tile([128, C], bf16, name="scr", tag="scr")
            nc.vector.memset(scr[:], 0.0)
            for n in WARMUP_NS:
                nc.tensor.matmul(psb[(0, 0)][:, 0:n], scr[:, 0:128], scr[:, 0:n],
                                 start=True, stop=True, skip_group_check=True)

            # --- matmul emission helpers ------------------------------------
            def lhsT(s, b, c):
                if c < 9:
                    dy, dx = TAPS[c]
                    v = Ta[s][:].rearrange("p (r q) -> p r q", q=PW)
                    return v[:, 4 * b + dy : 4 * b + dy + 4, dx : dx + 32]
                if c < 12:
                    j = c - 9
                    v = T34[s][:].rearrange("p (r q) -> p r q", q=PW)
                    return v[:, 4 * b : 4 * b + 4, j : j + 32]
                v = T1[s][:].rearrange("p (r q) -> p r q", q=PW)
                if c == 12:
                    return v[:, 4 * b + 2 : 4 * b + 6, 0:32]
                return v[0:64, 4 * b + 2 : 4 * b + 6, 2:34]

            def rhs(s, c):
                if c == 13:
                    return Wt[s][0:64, c * C : (c + 1) * C]
                return Wt[s][:, c * C : (c + 1) * C]

            def mm(s, b, c):
                nc.tensor.matmul(psb[(s, b)][:], lhsT(s, b, c), rhs(s, c),
                                 start=(c == 0), stop=(c == NCH - 1))

            # --- eviction + output DMA --------------------------------------
            stages = {}

            def evict(s, b):
                if s == S - 1 and b == NB - 1:
                    # tail block: smallest possible chain — single-block
                    # eviction, then a single-block transfer on SP's HWDGE.
                    st2 = oev.tile([128, C], f32, name=f"st2_{s}_{b}", tag="st")
                    nc.vector.tensor_scalar_add(st2[:], psb[(s, b)][:], 0.0)
                    nc.sync.dma_start(out_d[s, :, b, :], st2[:])
                    return
                if s == S - 1 and b == NB - 2:
                    # penultimate block: own single-block DMA so the tail
                    # DMA's transfer is as small as possible.
                    st1 = oev.tile([128, C], f32, name=f"st1_{s}_{b}", tag="st")
                    nc.scalar.copy(st1[:], psb[(s, b)][:])
                    nc.gpsimd.dma_start(out_d[s, :, b, :], st1[:])
                    return
                i = b // 2
                if b % 2 == 0:
                    stages[(s, i)] = oev.tile([128, 2 * C], f32,
                                              name=f"st_{s}_{i}", tag="st")
                dst = stages[(s, i)][:, (b % 2) * C : (b % 2 + 1) * C]
                if b % 2 == 0:
                    nc.vector.tensor_scalar_add(dst, psb[(s, b)][:], 0.0)
                else:
                    nc.scalar.copy(dst, psb[(s, b)][:])
                if b % 2 == 1:
                    st = stages[(s, i)][:].rearrange("p (b m) -> p b m", m=C)
                    nc.gpsimd.dma_start(out_d[s, :, 2 * i : 2 * i + 2, :], st)

            # --- sample 0: chunk-sweep order (matches DMA arrival) ----------
            for c in range(4):            # needs W c0-c3 + Ta rows 0-21
                for b in range(5):
                    mm(0, b, c)
            for c in range(4, 9):         # needs W c4-c8
                for b in range(5):
                    mm(0, b, c)
            for b in range(5, NB):        # needs Ta rows 22-33
                for c in range(9):
                    mm(0, b, c)
            for b in range(NB):           # needs T34, T1, W c9-c13
                for c in range(9, NCH):
                    mm(0, b, c)
                evict(0, b)

            # --- sample 1: block-major ------------------------------------
            for b in range(NB):
                for c in range(NCH):
                    mm(1, b, c)
                evict(1, b)

    nc.compile()
    return nc


def get_module():
    if "nc" not in _cache:
        _cache["nc"] = _build_module()
    return _cache["nc"]


def _route(x, gate_w, gate_b):
    """Replicates the reference router in numpy fp32. Returns combine [B,E]."""
    pooled = x.mean(axis=(2, 3), dtype=np.float32)
    logits = pooled @ gate_w + gate_b
    z = logits - logits.max(axis=-1, keepdims=True)
    ez = np.exp(z)
    w = ez / ez.sum(axis=-1, keepdims=True)
    topi = np.argsort(-w, axis=-1, kind="stable")[:, :TOPK]
    topw = np.take_along_axis(w, topi, axis=-1)
    topw = topw / (topw.sum(-1, keepdims=True) + 1e-10)
    combine = np.zeros((B, E), np.float32)
    np.put_along_axis(combine, topi, topw, axis=-1)
    return combine


def make_in_maps(x, gate_w, gate_b, expert_w, expert_b):
    import ml_dtypes

    bf16 = ml_dtypes.bfloat16
    x = np.ascontiguousarray(np.asarray(x, np.float32))
    gate_w = np.asarray(gate_w, np.float32)
    gate_b = np.asarray(gate_b, np.float32)
    expert_w = np.asarray(expert_w, np.float32)
    expert_b = np.asarray(expert_b, np.float32)

    combine = _route(x, gate_w, gate_b)                       # [B,E]
    Wc = np.einsum("be,eoikl->boikl", combine, expert_w)      # [B,C,C,3,3]
    bc = combine @ expert_b                                   # [B,C]

    # Padded input images: [B, C, 34*34]
    xp = np.zeros((B, C, PW, PW), np.float32)
    xp[:, :, 1 : H + 1, 1 : W + 1] = x
    xp = xp.reshape(B, C, PP).astype(bf16)

    ta = xp[:, 0:128]                                         # [B,128,1156]
    img64 = xp[:, 128:192]                                    # [B,64,1156]

    # ch128-191 duplicated with the upper half pre-shifted (d=34 / d=1)
    tb34 = np.zeros((B, 128, PP), bf16)
    tb34[:, 0:64] = img64
    tb34[:, 64:128, 0 : PP - PW] = img64[:, :, PW:]
    tb1 = np.zeros((B, 128, PP), bf16)
    tb1[:, 0:64] = img64
    tb1[:, 64:128, 0 : PP - 1] = img64[:, :, 1:]

    # Moving weights, one [128, 192] slab per K-chunk:
    #   WT[b, t, i, o] = Wc[b, o, i, dy, dx]
    WT = Wc.transpose(0, 3, 4, 2, 1).reshape(B, 9, C, C)
    w = np.zeros((B, 128, NCH * C), np.float32)
    for c in range(9):                        # ch 0-127, tap c
        w[:, :, c * C : (c + 1) * C] = WT[:, c, 0:128, :]
    for j in range(3):                        # pairs (0,j)+(1,j), d=34
        c = 9 + j
        w[:, 0:64, c * C : (c + 1) * C] = WT[:, j, 128:192, :]
        w[:, 64:128, c * C : (c + 1) * C] = WT[:, 3 + j, 128:192, :]
    w[:, 0:64, 12 * C : 13 * C] = WT[:, 6, 128:192, :]        # pair (2,0)+(2,1), d=1
    w[:, 64:128, 12 * C : 13 * C] = WT[:, 7, 128:192, :]
    w[:, 0:64, 13 * C : 14 * C] = WT[:, 8, 128:192, :]        # single (2,2), K=64
    w = w.astype(bf16)

    in_maps = []
    for cidx in range(NCORES):
        b0 = S * cidx
        in_maps.append(
            {
                "ta": np.ascontiguousarray(ta[b0 : b0 + S]),
                "tb34": np.ascontiguousarray(tb34[b0 : b0 + S]),
                "tb1": np.ascontiguousarray(tb1[b0 : b0 + S]),
                "w": np.ascontiguousarray(w[b0 : b0 + S]),
            }
        )
    return in_maps, bc


def postprocess(dev_out, bc_rows):
    """[S, 128, NB, 192] device tensor + per-sample bias rows -> [S,C,H,W]."""
    dev = np.asarray(dev_out, np.float32)
    out = dev.transpose(0, 3, 2, 1).reshape(S, C, NB, 4, 32).reshape(S, C, H, W)
    return out + bc_rows[:, :, None, None]


def kernel(x, gate_w, gate_b, expert_w, expert_b):
    from concourse.bass_utils import run_bass_kernel_spmd

    nc = get_module()
    in_maps, bc = make_in_maps(x, gate_w, gate_b, expert_w, expert_b)
    res = run_bass_kernel_spmd(nc, in_maps, core_ids=list(range(NCORES)))
    out = np.empty((B, C, H, W), np.float32)
    for c in range(NCORES):
        b0 = S * c
        out[b0 : b0 + S] = postprocess(res.results[c]["out"], bc[b0 : b0 + S])
    return out
